# revision 1
# baseline (speedup 1.0000x reference)
"""Overlapping-windows kernel (tf.nn.conv1d with identity filter) for TRN2.

Full input x: [64, 2000, 26] f32. Full output: [64, 2000, 494] f32 where
out[b, t, w*26 + c] = x_pad[b, t + w, c]  (x zero-padded by 9 frames each side).

Sharding: pure data parallel over batch — 8 examples per NeuronCore, 8 cores.

Per-core kernel (x_shard [8, 2000, 26] -> y_shard [8, 2000, 494]):
  Key observation: out[b, t, :] = x[b, t-9 : t+10, :].flatten() — each output
  row is a CONTIGUOUS 494-float slice of x[b] (row pitch 26 floats).

  Stage 1 (load): partition p = e*16 + k holds input rows
  [k*125-9, k*125+134) of example e (125 output rows + 9-row halos),
  flattened to 3718 floats; out-of-range halos zeroed by memset. Loads are
  per-example DMAs split across both HWDGE rings (sync + scalar).
  (SBUF-side DMA access patterns must keep ap[0] as the partition dim with
  step == row pitch; leading dims that hop multiple partitions silently
  corrupt addressing on HW.)

  Stage 2 (expand): DVE expands the 19 overlapping windows per output row
  into contiguous per-partition runs — one fused 4-dim-AP tensor_copy per
  chunk, 6 uneven chunks (small first chunk so the store pipe starts early)
  rotating through 3 buffers.

  Stage 3 (store): per chunk, one DMA writes [128 partitions x contiguous
  run] to y — big descriptors run at HBM line rate (a direct
  overlapping-window DMA with 1976B descriptors is ~2.4x slower per byte).
  Chunks alternate between the two HWDGE rings. WAR reuse of each buffer is
  gated by a per-buffer semaphore (a shared semaphore cannot distinguish
  WHICH of two outstanding DMAs completed).

  HBM traffic per core: 1.7 MB read + 31.6 MB write. Measured ~110-124 us
  (vs ~88 us write roofline; ~168 us for the no-expansion direct DMA).
"""

from contextlib import ExitStack

import numpy as np

import concourse.bass as bass
import concourse.mybir as mybir
from concourse.bass_utils import run_bass_kernel_spmd

# Problem constants (hardcoded per contract)
B_FULL = 64
T = 2000
C = 26
NCTX = 9
W = 2 * NCTX + 1          # 19
WC = W * C                # 494
N_CORES = 8
BL = B_FULL // N_CORES    # 8 examples per core
K = 16                    # row-chunks per example -> BL*K = 128 partitions
R = T // K                # 125 output rows per partition
FL = (R + 2 * NCTX) * C   # 3718 floats per partition (125+18 rows * 26)
HALO = NCTX * C           # 234 floats of halo on each side
XROW = T * C              # 52000 floats per example in x
YROW = T * WC             # 988000 floats per example in y
F32 = mybir.dt.float32

CHUNKS = (5, 24, 24, 24, 24, 24)  # output rows per expansion chunk
NBUF = 3                          # expansion ping-pong buffers


def _build():
    nchunk = len(CHUNKS)
    outw = max(CHUNKS) * WC
    starts = [sum(CHUNKS[:i]) for i in range(nchunk)]
    nc = bass.Bass()
    x = nc.dram_tensor("x", [BL, T, C], F32, kind="ExternalInput")
    y = nc.dram_tensor("y", [BL, T, WC], F32, kind="ExternalOutput")

    with ExitStack() as ctx:
        tile = ctx.enter_context(nc.sbuf_tensor("tile", [128, FL], F32))
        obufs = [ctx.enter_context(
                     nc.sbuf_tensor(f"obuf{i}", [128, outw], F32))
                 for i in range(NBUF)]
        vsem = ctx.enter_context(nc.semaphore("vsem"))
        lsemA = ctx.enter_context(nc.semaphore("lsemA"))
        lsemB = ctx.enter_context(nc.semaphore("lsemB"))
        esem = ctx.enter_context(nc.semaphore("esem"))
        osems = [ctx.enter_context(nc.semaphore(f"osem{i}"))
                 for i in range(NBUF)]
        block = ctx.enter_context(nc.Block())
        th = tile[:].tensor
        xt = x[:].tensor

        def half_loads(eng, es, lsem):
            for e in es:
                # interior chunks k=1..14: 14 contiguous partitions
                src = bass.AP(tensor=xt, offset=e * XROW + R * C - HALO,
                              ap=[[R * C, K - 2], [1, FL]])
                dst = bass.AP(tensor=th, offset=(e * K + 1) * FL,
                              ap=[[FL, K - 2], [1, FL]])
                eng.dma_start(out=dst, in_=src).then_inc(lsem, 16)
                # k=0: rows [0,134) -> partition e*16, cols [234, 3718)
                src0 = bass.AP(tensor=xt, offset=e * XROW,
                               ap=[[1, FL - HALO]])
                dst0 = bass.AP(tensor=th, offset=(e * K) * FL + HALO,
                               ap=[[FL, 1], [1, FL - HALO]])
                eng.dma_start(out=dst0, in_=src0).then_inc(lsem, 16)
                # k=15: rows [1866,2000) -> partition e*16+15, cols [0,3484)
                src15 = bass.AP(tensor=xt,
                                offset=e * XROW + (K - 1) * R * C - HALO,
                                ap=[[1, FL - HALO]])
                dst15 = bass.AP(tensor=th, offset=(e * K + K - 1) * FL,
                                ap=[[FL, 1], [1, FL - HALO]])
                eng.dma_start(out=dst15, in_=src15).then_inc(lsem, 16)

        def out_dma(eng, c):
            ob = obufs[c % NBUF][:].tensor
            cn = CHUNKS[c]
            src = bass.AP(tensor=ob, offset=0, ap=[[outw, 128], [1, cn * WC]])
            dst = bass.AP(tensor=y[:].tensor, offset=starts[c] * WC,
                          ap=[[R * WC, 128], [1, cn * WC]])
            eng.dma_start(out=dst, in_=src).then_inc(osems[c % NBUF], 16)

        @block.vector
        def _(vector):
            # Zero halo columns on all partitions (engines need aligned
            # start partitions); loads then overwrite non-halo spans.
            vector.memset(tile[:, 0:HALO], 0.0).then_inc(vsem, 1)
            vector.memset(tile[:, FL - HALO:FL], 0.0).then_inc(vsem, 1)
            vector.wait_ge(lsemA, 16 * 12)
            vector.wait_ge(lsemB, 16 * 12)
            for c in range(nchunk):
                if c >= NBUF:
                    # WAR: all prior out-DMAs of this buffer completed.
                    # Sound because this wait serializes per-buffer DMAs.
                    vector.wait_ge(osems[c % NBUF], 16 * (c // NBUF))
                ob = obufs[c % NBUF][:].tensor
                cn = CHUNKS[c]
                # ob[p, t*494 + w*26 + cc] = tile[p, (start + t + w)*26 + cc]
                src = bass.AP(tensor=th, offset=starts[c] * C,
                              ap=[[FL, 128], [C, cn], [C, W], [1, C]])
                dst = bass.AP(tensor=ob, offset=0,
                              ap=[[outw, 128], [WC, cn], [C, W], [1, C]])
                vector.tensor_copy(out=dst, in_=src).then_inc(esem, 1)

        @block.sync
        def _(sync):
            sync.wait_ge(vsem, 2)
            half_loads(sync, range(0, BL, 2), lsemA)
            for c in range(0, nchunk, 2):
                sync.wait_ge(esem, c + 1)
                out_dma(sync, c)
            for b in range(NBUF):
                ntot = len([c for c in range(nchunk) if c % NBUF == b])
                sync.wait_ge(osems[b], 16 * ntot)

        @block.scalar
        def _(scalar):
            scalar.wait_ge(vsem, 2)
            half_loads(scalar, range(1, BL, 2), lsemB)
            for c in range(1, nchunk, 2):
                scalar.wait_ge(esem, c + 1)
                out_dma(scalar, c)

    return nc


_NC = None


def _get_nc():
    global _NC
    if _NC is None:
        _NC = _build()
    return _NC


def run(x: np.ndarray, trace: bool = False):
    """Run the kernel on all 8 cores; returns (y_full, BassKernelResults)."""
    x = np.ascontiguousarray(x, dtype=np.float32)
    assert x.shape == (B_FULL, T, C), x.shape
    nc = _get_nc()
    in_maps = [
        {"x": x[i * BL:(i + 1) * BL]} for i in range(N_CORES)
    ]
    res = run_bass_kernel_spmd(
        nc, in_maps, core_ids=list(range(N_CORES)), trace=trace
    )
    y = np.concatenate([res.results[i]["y"] for i in range(N_CORES)], axis=0)
    return y, res


def kernel(x: np.ndarray) -> np.ndarray:
    y, _ = run(x)
    return y



# revision 2
# speedup vs baseline: 1.0083x; 1.0083x over previous
"""Overlapping-windows kernel (tf.nn.conv1d with identity filter) for TRN2.

Full input x: [64, 2000, 26] f32. Full output: [64, 2000, 494] f32 where
out[b, t, w*26 + c] = x_pad[b, t + w, c]  (x zero-padded by 9 frames each side).

Sharding: pure data parallel over batch — 8 examples per NeuronCore, 8 cores.

Per-core kernel (x_shard [8, 2000, 26] -> y_shard [8, 2000, 494]):
  Partition p = e*16 + k holds input rows [k*125-9, k*125+134) of example e
  (125 output rows + 9-row halos) flattened to 3718 floats. Because
  XROW == 16*RC and YROW == 16*R*WC, BOTH x and y addresses are uniform-stride
  across all 128 partitions — whole-grid single-DMA loads/stores.

  Loads (overlapped under stores, gated per column-slice):
    D1 [234,832) on sync, DB [832,2080) on scalar, DC [2080,3484) on gpsimd
    (SWDGE, own queue — never head-of-line blocks the store rings), plus
    per-example edge DMAs (left halo cols [0,234) k>=1, right edge
    [3484,3718) k<=14) trickled on gpsimd. Halo zeros via 2 vector memsets.

  Expand: DVE tensor_copy per chunk (fused 4-dim AP), 2 elem/cycle f32
  (2x_2P mode). Chunks REORDERED so the left-halo-dependent rows [0,10)
  expand LAST: chunk c only waits for the loads covering its columns, so
  the first store issues ~12us instead of ~29us (loads were the baseline's
  serial bottleneck: 19us issue+drain before any expand).

  Stores: one DMA per chunk, [128 partitions x contiguous run] — big
  descriptors at HBM line rate. 10 chunks alternate rings, byte-balanced
  (63/62 rows) so both rings drain together. 5 rotating buffers; WAR reuse
  gated by per-buffer semaphores.

  HBM traffic per core: ~2 MB read + 31.6 MB write. Store phase measured
  ~405 GB/s aggregate; roofline ~90us incl fixed ~7us NEFF preamble.
"""

from contextlib import ExitStack

import numpy as np

import concourse.bass as bass
import concourse.mybir as mybir
from concourse.bass_utils import run_bass_kernel_spmd

# Problem constants (hardcoded per contract)
B_FULL = 64
T = 2000
C = 26
NCTX = 9
W = 2 * NCTX + 1          # 19
WC = W * C                # 494
N_CORES = 8
BL = B_FULL // N_CORES    # 8 examples per core
K = 16                    # row-chunks per example -> BL*K = 128 partitions
R = T // K                # 125 output rows per partition
RC = R * C                # 3250
FL = (R + 2 * NCTX) * C   # 3718 floats per partition (125+18 rows * 26)
HALO = NCTX * C           # 234 floats of halo on each side
XROW = T * C              # 52000 floats per example in x
RWC = R * WC              # 61750 floats per partition-row in y
F32 = mybir.dt.float32

# (start_row, n_rows, ring) — ring 0 = sync, 1 = scalar. Left-halo rows
# [0,10) go LAST (only they need the per-example edge loads). Ring rows:
# sync 3+12+16+16+16=63, scalar 12+12+12+16+10=62.
CHUNKS = [
    (10, 3, 0), (13, 12, 1), (25, 12, 1), (37, 12, 0), (49, 12, 1),
    (61, 16, 0), (77, 16, 1), (93, 16, 0), (109, 16, 0), (0, 10, 1),
]
NBUF = 5
OUTW = max(cn for _, cn, _ in CHUNKS) * WC

# Column slices of the tile covered by each bulk load DMA.
D1_C0, D1_C1 = HALO, 832        # sync; needed by chunk 0 (cols >= 260)
DB_C0, DB_C1 = 832, 2080        # scalar; needed from chunk 1 (cols < 1118)
DC_C0, DC_C1 = 2080, 3484       # gpsimd; needed from chunk 5 (cols < 2470)


def _build():
    nchunk = len(CHUNKS)
    nc = bass.Bass()
    x = nc.dram_tensor("x", [BL, T, C], F32, kind="ExternalInput")
    y = nc.dram_tensor("y", [BL, T, WC], F32, kind="ExternalOutput")

    with ExitStack() as ctx:
        tile = ctx.enter_context(nc.sbuf_tensor("tile", [128, FL], F32))
        obufs = [ctx.enter_context(
                     nc.sbuf_tensor(f"obuf{i}", [128, OUTW], F32))
                 for i in range(NBUF)]
        vsem = ctx.enter_context(nc.semaphore("vsem"))
        lsemA = ctx.enter_context(nc.semaphore("lsemA"))
        lsemB = ctx.enter_context(nc.semaphore("lsemB"))
        lsemC = ctx.enter_context(nc.semaphore("lsemC"))
        lsemD = ctx.enter_context(nc.semaphore("lsemD"))
        lsemE = ctx.enter_context(nc.semaphore("lsemE"))
        esem = ctx.enter_context(nc.semaphore("esem"))
        osems = [ctx.enter_context(nc.semaphore(f"osem{i}"))
                 for i in range(NBUF)]
        block = ctx.enter_context(nc.Block())
        th = tile[:].tensor
        xt = x[:].tensor
        yt = y[:].tensor

        def bulk_load(eng, c0, c1, lsem):
            # All 128 partitions, tile cols [c0, c1); x is uniform stride RC
            # across partitions (p*RC + col - HALO).
            n = c1 - c0
            src = bass.AP(tensor=xt, offset=c0 - HALO, ap=[[RC, 128], [1, n]])
            dst = bass.AP(tensor=th, offset=c0, ap=[[FL, 128], [1, n]])
            eng.dma_start(out=dst, in_=src).then_inc(lsem, 16)

        def out_dma(eng, c):
            start, cn, _ = CHUNKS[c]
            ob = obufs[c % NBUF][:].tensor
            src = bass.AP(tensor=ob, offset=0, ap=[[OUTW, 128], [1, cn * WC]])
            dst = bass.AP(tensor=yt, offset=start * WC,
                          ap=[[RWC, 128], [1, cn * WC]])
            eng.dma_start(out=dst, in_=src).then_inc(osems[c % NBUF], 16)

        @block.vector
        def _(vector):
            # Zero halos: left halo cols (only k==0 partitions keep them;
            # the per-example left-edge loads overwrite k>=1) and right
            # edge (k==15 partitions keep; right-edge loads overwrite
            # k<=14). Gates the gpsimd edge loads via vsem.
            vector.memset(tile[:, 0:HALO], 0.0).then_inc(vsem, 1)
            vector.memset(tile[:, FL - HALO:FL], 0.0).then_inc(vsem, 1)
            nuse = [0] * NBUF
            for c in range(nchunk):
                start, cn, _ = CHUNKS[c]
                if c == 0:
                    vector.wait_ge(lsemA, 16)
                elif c == 1:
                    vector.wait_ge(lsemB, 16)
                elif c == 5:
                    vector.wait_ge(lsemC, 16)
                elif c == 8:
                    vector.wait_ge(lsemD, 16 * BL)
                elif c == nchunk - 1:
                    vector.wait_ge(lsemE, 16 * BL)
                b = c % NBUF
                if c >= NBUF:
                    # WAR: all prior out-DMAs of this buffer completed.
                    vector.wait_ge(osems[b], 16 * nuse[b])
                nuse[b] += 1
                ob = obufs[b][:].tensor
                # ob[p, t*494 + w*26 + cc] = tile[p, (start + t + w)*26 + cc]
                src = bass.AP(tensor=th, offset=start * C,
                              ap=[[FL, 128], [C, cn], [C, W], [1, C]])
                dst = bass.AP(tensor=ob, offset=0,
                              ap=[[OUTW, 128], [WC, cn], [C, W], [1, C]])
                vector.tensor_copy(out=dst, in_=src).then_inc(esem, 1)

        @block.sync
        def _(sync):
            bulk_load(sync, D1_C0, D1_C1, lsemA)
            for c in range(nchunk):
                if CHUNKS[c][2] == 0:
                    sync.wait_ge(esem, c + 1)
                    out_dma(sync, c)
            # All stores (both rings) complete before the NEFF retires.
            for b in range(NBUF):
                ntot = len([c for c in range(nchunk) if c % NBUF == b])
                sync.wait_ge(osems[b], 16 * ntot)

        @block.scalar
        def _(scalar):
            bulk_load(scalar, DB_C0, DB_C1, lsemB)
            for c in range(nchunk):
                if CHUNKS[c][2] == 1:
                    scalar.wait_ge(esem, c + 1)
                    out_dma(scalar, c)

        @block.gpsimd
        def _(gpsimd):
            bulk_load(gpsimd, DC_C0, DC_C1, lsemC)
            gpsimd.wait_ge(vsem, 2)
            for e in range(BL):
                # Right edge: k=0..14, cols [3484,3718) <- x[e] rows
                # [(k+1)*125, (k+1)*125+9).
                src = bass.AP(tensor=xt, offset=e * XROW + RC,
                              ap=[[RC, K - 1], [1, HALO]])
                dst = bass.AP(tensor=th, offset=(e * K) * FL + (FL - HALO),
                              ap=[[FL, K - 1], [1, HALO]])
                gpsimd.dma_start(out=dst, in_=src).then_inc(lsemD, 16)
            for e in range(BL):
                # Left halo: k=1..15, cols [0,234) <- x[e] rows
                # [k*125-9, k*125).
                src = bass.AP(tensor=xt, offset=e * XROW + RC - HALO,
                              ap=[[RC, K - 1], [1, HALO]])
                dst = bass.AP(tensor=th, offset=(e * K + 1) * FL,
                              ap=[[FL, K - 1], [1, HALO]])
                gpsimd.dma_start(out=dst, in_=src).then_inc(lsemE, 16)

    return nc


_NC = None


def _get_nc():
    global _NC
    if _NC is None:
        _NC = _build()
    return _NC


def run(x: np.ndarray, trace: bool = False):
    """Run the kernel on all 8 cores; returns (y_full, BassKernelResults)."""
    x = np.ascontiguousarray(x, dtype=np.float32)
    assert x.shape == (B_FULL, T, C), x.shape
    nc = _get_nc()
    in_maps = [
        {"x": x[i * BL:(i + 1) * BL]} for i in range(N_CORES)
    ]
    res = run_bass_kernel_spmd(
        nc, in_maps, core_ids=list(range(N_CORES)), trace=trace
    )
    y = np.concatenate([res.results[i]["y"] for i in range(N_CORES)], axis=0)
    return y, res


def kernel(x: np.ndarray) -> np.ndarray:
    y, _ = run(x)
    return y


# revision 3
# speedup vs baseline: 1.2050x; 1.1951x over previous
"""Overlapping-windows kernel (tf.nn.conv1d with identity filter) for TRN2.

Full input x: [64, 2000, 26] f32. Full output: [64, 2000, 494] f32 where
out[b, t, w*26 + c] = x_pad[b, t + w, c]  (x zero-padded by 9 frames each side).

Sharding: data parallel over batch with HALO-OVERLAP — 8 examples per core,
and the per-core input is pre-tiled ON THE HOST into [128, 3718]: partition
p = e*16 + k holds zero-padded rows [k*125-9, k*125+134) of example e
(125 output rows + 9-row halos, flattened). Host cost: ~2 MB strided copy
per core. This makes the device side completely uniform: no edge-case DMAs,
no memsets, every load/store is a whole-grid 128-descriptor transfer.

Device pipeline (xt [128, 3718] -> y [8, 2000, 494], flat per-partition
row stride in y is RWC = 125*494 for ALL partitions since YROW == 16*RWC):

  Loads: 4 column-slice DMAs (one per slice, 128 big descriptors each),
  issued up front on the two HWDGE rings; each expand chunk gates only on
  the slices covering its columns, so the store pipe starts ~12us in.

  Expand: DVE tensor_copy per chunk (fused 4-dim AP, f32 2x_2P mode,
  ~2 elem/cycle): ob[p, i*494 + w*26 + c] = tile[p, (start+i+w)*26 + c].

  Store: one DMA per chunk, [128 partitions x contiguous run] — ~20 KB
  descriptors drain at HBM line rate (~405 GB/s aggregate measured).
  13 chunks over 8 rotating buffers (WAR via per-buffer semaphores), ring
  assignment byte-balanced (62/63 rows) so both rings finish together.

  Roofline: ~33.2 MB HBM traffic / ~0.40 GB/us + ~7us fixed NEFF preamble
  + ~2us teardown ~= low-90s us.
"""

from contextlib import ExitStack

import numpy as np

import concourse.bass as bass
import concourse.mybir as mybir
from concourse.bass_utils import run_bass_kernel_spmd

# Problem constants (hardcoded per contract)
B_FULL = 64
T = 2000
C = 26
NCTX = 9
W = 2 * NCTX + 1          # 19
WC = W * C                # 494
N_CORES = 8
BL = B_FULL // N_CORES    # 8 examples per core
K = 16                    # row-chunks per example -> BL*K = 128 partitions
R = T // K                # 125 output rows per partition
RC = R * C                # 3250
FL = (R + 2 * NCTX) * C   # 3718 floats per partition (125+18 rows * 26)
RWC = R * WC              # 61750 floats per partition-row in y
F32 = mybir.dt.float32

SIZES = [4, 8, 10, 10, 10, 10, 10, 10, 10, 10, 11, 11, 11]   # 125 rows
STARTS = [sum(SIZES[:i]) for i in range(len(SIZES))]
# ring 0 = sync, 1 = scalar; byte-balanced: sync 62 rows, scalar 63.
RINGS = [1, 1, 0, 1, 0, 1, 0, 1, 0, 1, 0, 1, 0]
NBUF = 8
OUTW = max(SIZES) * WC

# Column slices loaded by each bulk DMA; chunk c gates on the slice that
# completes its columns: c0<-LA, c1<-LB, c6<-LC1, c10<-LC2.
SLICES = [(0, 572), (572, 1976), (1976, 2886), (2886, 3718)]
SLICE_RING = [0, 1, 0, 0]      # LA,LC1,LC2 on sync; LB on scalar
CHUNK_GATE = {0: 0, 1: 1, 6: 2, 10: 3}   # chunk -> slice index


def _build():
    nchunk = len(SIZES)
    nc = bass.Bass()
    xt = nc.dram_tensor("xt", [128, FL], F32, kind="ExternalInput")
    y = nc.dram_tensor("y", [BL, T, WC], F32, kind="ExternalOutput")

    with ExitStack() as ctx:
        tile = ctx.enter_context(nc.sbuf_tensor("tile", [128, FL], F32))
        obufs = [ctx.enter_context(
                     nc.sbuf_tensor(f"obuf{i}", [128, OUTW], F32))
                 for i in range(NBUF)]
        lsems = [ctx.enter_context(nc.semaphore(f"lsem{i}"))
                 for i in range(len(SLICES))]
        esem = ctx.enter_context(nc.semaphore("esem"))
        osems = [ctx.enter_context(nc.semaphore(f"osem{i}"))
                 for i in range(NBUF)]
        block = ctx.enter_context(nc.Block())
        th = tile[:].tensor
        xtt = xt[:].tensor
        yt = y[:].tensor

        def bulk_load(eng, s):
            c0, c1 = SLICES[s]
            n = c1 - c0
            src = bass.AP(tensor=xtt, offset=c0, ap=[[FL, 128], [1, n]])
            dst = bass.AP(tensor=th, offset=c0, ap=[[FL, 128], [1, n]])
            eng.dma_start(out=dst, in_=src).then_inc(lsems[s], 16)

        def out_dma(eng, c):
            start, cn = STARTS[c], SIZES[c]
            ob = obufs[c % NBUF][:].tensor
            src = bass.AP(tensor=ob, offset=0, ap=[[OUTW, 128], [1, cn * WC]])
            dst = bass.AP(tensor=yt, offset=start * WC,
                          ap=[[RWC, 128], [1, cn * WC]])
            eng.dma_start(out=dst, in_=src).then_inc(osems[c % NBUF], 16)

        @block.vector
        def _(vector):
            nuse = [0] * NBUF
            for c in range(nchunk):
                if c in CHUNK_GATE:
                    vector.wait_ge(lsems[CHUNK_GATE[c]], 16)
                b = c % NBUF
                if c >= NBUF:
                    # WAR: all prior out-DMAs of this buffer completed.
                    vector.wait_ge(osems[b], 16 * nuse[b])
                nuse[b] += 1
                start, cn = STARTS[c], SIZES[c]
                ob = obufs[b][:].tensor
                # ob[p, i*494 + w*26 + cc] = tile[p, (start + i + w)*26 + cc]
                src = bass.AP(tensor=th, offset=start * C,
                              ap=[[FL, 128], [C, cn], [C, W], [1, C]])
                dst = bass.AP(tensor=ob, offset=0,
                              ap=[[OUTW, 128], [WC, cn], [C, W], [1, C]])
                vector.tensor_copy(out=dst, in_=src).then_inc(esem, 1)

        @block.sync
        def _(sync):
            for s in range(len(SLICES)):
                if SLICE_RING[s] == 0:
                    bulk_load(sync, s)
            for c in range(nchunk):
                if RINGS[c] == 0:
                    sync.wait_ge(esem, c + 1)
                    out_dma(sync, c)
            # All stores (both rings) complete before the NEFF retires.
            for b in range(NBUF):
                ntot = len([c for c in range(nchunk) if c % NBUF == b])
                sync.wait_ge(osems[b], 16 * ntot)

        @block.scalar
        def _(scalar):
            for s in range(len(SLICES)):
                if SLICE_RING[s] == 1:
                    bulk_load(scalar, s)
            for c in range(nchunk):
                if RINGS[c] == 1:
                    scalar.wait_ge(esem, c + 1)
                    out_dma(scalar, c)

    return nc


_NC = None


def _get_nc():
    global _NC
    if _NC is None:
        _NC = _build()
    return _NC


def _host_tile(x_core: np.ndarray) -> np.ndarray:
    """[8, 2000, 26] -> [128, 3718]: halo-overlapped, zero-padded row tiles."""
    xp = np.pad(x_core, ((0, 0), (NCTX, NCTX), (0, 0)))
    xpf = np.ascontiguousarray(xp).reshape(BL, -1)   # [8, 52468]
    st = xpf.strides
    tl = np.lib.stride_tricks.as_strided(
        xpf, shape=(BL, K, FL), strides=(st[0], RC * 4, 4))
    return np.ascontiguousarray(tl.reshape(128, FL))


def run(x: np.ndarray, trace: bool = False):
    """Run the kernel on all 8 cores; returns (y_full, BassKernelResults)."""
    x = np.ascontiguousarray(x, dtype=np.float32)
    assert x.shape == (B_FULL, T, C), x.shape
    nc = _get_nc()
    in_maps = [
        {"xt": _host_tile(x[i * BL:(i + 1) * BL])} for i in range(N_CORES)
    ]
    res = run_bass_kernel_spmd(
        nc, in_maps, core_ids=list(range(N_CORES)), trace=trace
    )
    y = np.concatenate([res.results[i]["y"] for i in range(N_CORES)], axis=0)
    return y, res


def kernel(x: np.ndarray) -> np.ndarray:
    y, _ = run(x)
    return y
